# revision 7
# baseline (speedup 1.0000x reference)
"""Trainium2 Bass kernel for CFGSubASTExpressionCombiner (segment-softmax
attention over sub-ASTs grouped by PDG node).

Contract: kernel(**inputs) takes FULL unsharded numpy inputs, returns the
FULL [N_PDG, D] output. Internally shards PDG segments across 8 NeuronCores.
Within a core, segments are bin-packed (LPT) into 49 blocks of <=128
segments each so per-block element counts are flat; per-block element-tile
counts are baked into the program (variable, data-dependent).

Math (per segment s with element set E_s, all on device):
    q_s   = ast[root(s)]
    qk_s  = q_s @ (Wk.T * scale)        (scale folded into host-passed WkT)
    S[e,s]= x_e . qk_s                  (dense per 128-seg block, via PE)
    P     = exp(S) * [seg(e)==s]        (fused is_equal*exp on DVE)
    [U|Z] = P.T @ [X|1]                 (segment sums via PE, PSUM accum)
    out_s = (U_s / max(Z_s,eps)) @ Wv
No max-subtraction: scores are ~N(0,1) (|s|<~6), exp is safe in fp32, and
softmax is shift-invariant so results match the reference to fp32 rounding.

Engine budget per element tile (128 elements):
    Pool : 1 indirect gather (~1us SWDGE, the bottleneck)
    PE   : 2 transposes + 2 S-chunks + 1 U-accum  (~770 col-cycles)
    DVE  : xT copy (256) + fused mask*exp (128)
    Act  : exp (128) - Act engine does ONLY Exp (no act-table thrash)
"""

import math

import numpy as np

import concourse.bass as bass
import concourse.bacc as bacc
import concourse.mybir as mybir
import concourse.tile as tile
from concourse.bass_utils import run_bass_kernel_spmd
from concourse.masks import make_identity

P = 128
D = 256
N_CORES = 8

# Full-problem constants (hardcoded per contract).
N_AST_FULL = 500000
N_PDG_FULL = 50000
SEGS_PER_CORE_FULL = N_PDG_FULL // N_CORES          # 6250
N_BLOCKS_FULL = math.ceil(SEGS_PER_CORE_FULL / P)   # 49

f32 = mybir.dt.float32
i32 = mybir.dt.int32
bf16 = mybir.dt.bfloat16

# x-path dtype: flip both to bf16 to halve gather bytes (validated vs
# the 2e-2 rel-err gate before enabling).
try:
    import ml_dtypes
    _NP_BF16 = ml_dtypes.bfloat16
except ImportError:
    _NP_BF16 = None
X_DT = f32
X_NP_DT = np.float32

EXP = mybir.ActivationFunctionType.Exp


def _build_nc(n_ast, tiles_per_block, mode="full", reps=1,
              xp_bufs=6, blk_bufs=3, xdt=f32):
    """One SPMD NeuronCore program. Sizes fixed at build time.

    tiles_per_block[b] = number of 128-element tiles in segment-block b.
    mode: "full" = real kernel; "gather" = gathers + tiny reduces only.
    reps: repeat the whole block loop (differential timing only).
    """
    n_blocks = len(tiles_per_block)
    tile_off = np.concatenate([[0], np.cumsum(tiles_per_block)]).astype(int)
    n_cols = int(tile_off[-1])
    seg_slots = n_blocks * P
    e_slots = n_cols * P

    nc = bacc.Bacc()
    ast = nc.declare_dram_parameter("ast", [n_ast, D], xdt, isOutput=False)
    wkt = nc.declare_dram_parameter("wkt", [D, D], xdt, isOutput=False)
    wv = nc.declare_dram_parameter("wv", [D, D], xdt, isOutput=False)
    gidx = nc.declare_dram_parameter("gidx", [e_slots], i32, isOutput=False)
    slid = nc.declare_dram_parameter("slid", [e_slots], xdt, isOutput=False)
    root = nc.declare_dram_parameter("root", [seg_slots], i32, isOutput=False)
    out = nc.declare_dram_parameter("out", [seg_slots, D], f32, isOutput=True)

    with tile.TileContext(nc) as tc:
        with (
            tc.tile_pool(name="const", bufs=1) as cpool,
            tc.tile_pool(name="blk", bufs=blk_bufs) as bpool,
            tc.tile_pool(name="xp", bufs=xp_bufs) as xpool,
            tc.tile_pool(name="pt", bufs=2, space="PSUM") as pt,
            tc.tile_pool(name="pu", bufs=2, space="PSUM") as pu,
        ):
            # Resident constants: Wk.T (pre-scaled) and Wv as two 128-row
            # K-chunks side by side; identity for PE transpose; iota row.
            wk2 = cpool.tile([P, 2 * D], xdt)
            nc.sync.dma_start(out=wk2[:, 0:D], in_=wkt[0:P, :])
            nc.sync.dma_start(out=wk2[:, D : 2 * D], in_=wkt[P : 2 * P, :])
            wv2 = cpool.tile([P, 2 * D], xdt)
            nc.sync.dma_start(out=wv2[:, 0:D], in_=wv[0:P, :])
            nc.sync.dma_start(out=wv2[:, D : 2 * D], in_=wv[P : 2 * P, :])
            ident = cpool.tile([P, P], xdt)
            make_identity(nc, ident[:])
            iota_i = cpool.tile([P, P], i32)
            nc.gpsimd.iota(iota_i[:], pattern=[[1, P]], base=0, channel_multiplier=0)
            iota_f = cpool.tile([P, P], xdt)
            nc.vector.tensor_copy(iota_f[:], iota_i[:])

            # All index arrays resident in SBUF, one DMA each: column c of
            # gx_all/sl_all is element-tile c (tile t of block b at
            # c = tile_off[b] + t); column b of root_all is segment block b.
            gx_all = cpool.tile([P, n_cols], i32)
            nc.sync.dma_start(
                out=gx_all[:], in_=gidx[:].rearrange("(p c) -> p c", c=n_cols)
            )
            sl_all = cpool.tile([P, n_cols], xdt)
            nc.sync.dma_start(
                out=sl_all[:], in_=slid[:].rearrange("(p c) -> p c", c=n_cols)
            )
            root_all = cpool.tile([P, n_blocks], i32)
            nc.sync.dma_start(
                out=root_all[:], in_=root[:].rearrange("(p b) -> p b", b=n_blocks)
            )

            for _rep in range(reps):
              for b in range(n_blocks):
                t_b = int(tiles_per_block[b])
                # ---- segment side: q rows -> qk^T (d on partitions) ----
                q = bpool.tile([P, D], xdt)
                nc.gpsimd.indirect_dma_start(
                    out=q[:],
                    out_offset=None,
                    in_=ast[:],
                    in_offset=bass.IndirectOffsetOnAxis(ap=root_all[:, b : b + 1], axis=0),
                )
                if mode == "gather":
                    acc = xpool.tile([P, 16], f32, tag="acc")
                    nc.vector.tensor_reduce(
                        acc[:, 15:16], q[:],
                        axis=mybir.AxisListType.X, op=mybir.AluOpType.max,
                    )
                    for t in range(t_b):
                        c = tile_off[b] + t
                        x = xpool.tile([P, D], xdt, tag="xg")
                        nc.gpsimd.indirect_dma_start(
                            out=x[:],
                            out_offset=None,
                            in_=ast[:],
                            in_offset=bass.IndirectOffsetOnAxis(
                                ap=gx_all[:, c : c + 1], axis=0
                            ),
                        )
                        nc.vector.tensor_reduce(
                            acc[:, t % 15 : t % 15 + 1], x[:],
                            axis=mybir.AxisListType.X, op=mybir.AluOpType.max,
                        )
                    nc.sync.dma_start(
                        out=out[b * P : (b + 1) * P, 0:16], in_=acc[:]
                    )
                    continue
                qT_ps = pt.tile([P, D], xdt, tag="tr")
                nc.tensor.transpose(qT_ps[:, 0:P], q[:, 0:P], ident[:])
                nc.tensor.transpose(qT_ps[:, P:D], q[:, P:D], ident[:])
                qT = bpool.tile([P, D], xdt)
                nc.vector.tensor_copy(qT[:], qT_ps[:])

                qkT_ps = pt.tile([P, D], f32, tag="mm")
                for m in range(2):
                    for k in range(2):
                        nc.tensor.matmul(
                            qkT_ps[:, m * P : (m + 1) * P],
                            lhsT=wk2[:, k * D + m * P : k * D + (m + 1) * P],
                            rhs=qT[:, k * P : (k + 1) * P],
                            start=(k == 0),
                            stop=(k == 1),
                        )
                qkT = bpool.tile([P, D], xdt)
                nc.vector.tensor_copy(qkT[:], qkT_ps[:])

                # ---- element side: accumulate [U | Z] over t_b tiles ----
                u_ps = pu.tile([P, D + 1], f32, tag="u")
                for t in range(t_b):
                    c = tile_off[b] + t
                    xt_ = xpool.tile([P, D + 1], xdt, tag="xg")
                    x = xt_[:]
                    nc.vector.memset(x[:, D : D + 1], 1.0)
                    nc.gpsimd.indirect_dma_start(
                        out=x[:, 0:D],
                        out_offset=None,
                        in_=ast[:],
                        in_offset=bass.IndirectOffsetOnAxis(
                            ap=gx_all[:, c : c + 1], axis=0
                        ),
                    )
                    xT_ps = pt.tile([P, D], xdt, tag="tr")
                    nc.tensor.transpose(xT_ps[:, 0:P], x[:, 0:P], ident[:])
                    nc.tensor.transpose(xT_ps[:, P:D], x[:, P:D], ident[:])
                    xT = xpool.tile([P, D], xdt)
                    nc.vector.tensor_copy(xT[:], xT_ps[:])

                    s_ps = pt.tile([P, P], f32, tag="s")
                    for k in range(2):
                        nc.tensor.matmul(
                            s_ps[:],
                            lhsT=xT[:, k * P : (k + 1) * P],
                            rhs=qkT[:, k * P : (k + 1) * P],
                            start=(k == 0),
                            stop=(k == 1),
                        )
                    ptil = xpool.tile([P, P], xdt)
                    nc.scalar.activation(ptil[:], s_ps[:], EXP)
                    pmat = xpool.tile([P, P], xdt)
                    nc.vector.scalar_tensor_tensor(
                        out=pmat[:],
                        in0=iota_f[:],
                        scalar=sl_all[:, c : c + 1],
                        in1=ptil[:],
                        op0=mybir.AluOpType.is_equal,
                        op1=mybir.AluOpType.mult,
                    )
                    nc.tensor.matmul(
                        u_ps[:],
                        lhsT=pmat[:],
                        rhs=x[:],
                        start=(t == 0),
                        stop=(t == t_b - 1),
                    )

                # ---- finalize block: (U/Z) @ Wv  (DVE only; Act stays Exp) --
                z = bpool.tile([P, 1], f32)
                nc.vector.tensor_scalar_max(z[:], u_ps[:, D : D + 1], 1e-30)
                rz = bpool.tile([P, 1], f32)
                nc.vector.reciprocal(rz[:], z[:])
                up = bpool.tile([P, D], xdt)
                nc.vector.tensor_scalar_mul(up[:], u_ps[:, 0:D], rz[:, 0:1])
                upT_ps = pt.tile([P, D], xdt, tag="tr")
                nc.tensor.transpose(upT_ps[:, 0:P], up[:, 0:P], ident[:])
                nc.tensor.transpose(upT_ps[:, P:D], up[:, P:D], ident[:])
                upT = bpool.tile([P, D], xdt)
                nc.vector.tensor_copy(upT[:], upT_ps[:])
                f_ps = pt.tile([P, D], f32, tag="mm")
                for k in range(2):
                    nc.tensor.matmul(
                        f_ps[:],
                        lhsT=upT[:, k * P : (k + 1) * P],
                        rhs=wv2[:, k * D : (k + 1) * D],
                        start=(k == 0),
                        stop=(k == 1),
                    )
                o = bpool.tile([P, D], f32)
                nc.vector.tensor_copy(o[:], f_ps[:])
                nc.sync.dma_start(out=out[b * P : (b + 1) * P, :], in_=o[:])
    nc.finalize()
    return nc


_NC_CACHE = {}


def _get_nc(n_ast, tiles_per_block, mode="full", reps=1, xp_bufs=6,
            blk_bufs=3, xdt=f32):
    key = (n_ast, tuple(tiles_per_block), mode, reps, xp_bufs, blk_bufs,
           str(xdt))
    if key not in _NC_CACHE:
        _NC_CACHE[key] = _build_nc(
            n_ast, list(tiles_per_block), mode=mode, reps=reps,
            xp_bufs=xp_bufs, blk_bufs=blk_bufs, xdt=xdt,
        )
    return _NC_CACHE[key]


def _binpack_core(counts_core, n_blocks):
    """LPT bin-packing of segments into n_blocks blocks of <=128 segs.

    Returns (blocks, loads): blocks = list of lists of local segment ids,
    loads = element count per block. Blocks sorted by load descending so
    block index -> load rank is aligned across cores.
    """
    import heapq

    n_seg = len(counts_core)
    order = np.argsort(-counts_core, kind="stable")
    heap = [(0, b) for b in range(n_blocks)]
    heapq.heapify(heap)
    blocks = [[] for _ in range(n_blocks)]
    loads = np.zeros(n_blocks, dtype=np.int64)
    deferred = []
    for s in order:
        load, b = heapq.heappop(heap)
        blocks[b].append(s)
        loads[b] = load + counts_core[s]
        if len(blocks[b]) < P:
            heapq.heappush(heap, (loads[b], b))
        else:
            deferred.append(b)
    # sort blocks by load desc for cross-core alignment
    border = np.argsort(-loads, kind="stable")
    blocks = [blocks[i] for i in border]
    loads = loads[border]
    return blocks, loads


def prepare_in_maps(
    ast_np, wkt_s, wv_np, ast_to_pdg_key, ast_to_pdg_value,
    pdg_to_root_key, pdg_to_root_value, n_pdg,
    segs_per_core=None, n_blocks=None, x_np_dt=None,
):
    """Host-side prep: sort elements by segment, bin-pack segments into
    blocks (per core), pad, build per-core in_maps.

    Returns (in_maps, meta) where meta["tiles_per_block"] parameterizes the
    program and meta["out_seg"] maps per-core out rows -> global segment ids.
    """
    n_ast = ast_np.shape[0]
    if x_np_dt is None:
        x_np_dt = X_NP_DT
    if ast_np.dtype != x_np_dt:
        ast_np = np.ascontiguousarray(ast_np.astype(x_np_dt))
        wkt_s = np.ascontiguousarray(wkt_s.astype(x_np_dt))
        wv_np = np.ascontiguousarray(wv_np.astype(x_np_dt))
    if segs_per_core is None:
        segs_per_core = (n_pdg + N_CORES - 1) // N_CORES
    if n_blocks is None:
        n_blocks = math.ceil(segs_per_core / P)

    order = np.argsort(ast_to_pdg_value, kind="stable")
    seg_sorted = np.asarray(ast_to_pdg_value)[order]
    gid_sorted = np.asarray(ast_to_pdg_key)[order].astype(np.int32)
    counts = np.bincount(seg_sorted, minlength=n_pdg).astype(np.int64)
    cum = np.concatenate([[0], np.cumsum(counts)]).astype(np.int64)

    root_full = np.zeros(n_pdg, dtype=np.int32)
    root_full[np.asarray(pdg_to_root_key)] = np.asarray(pdg_to_root_value)

    # Per-core bin-packing; then per-block-index max tiles across cores
    # (the SPMD program is shared by all cores).
    core_blocks = []
    core_loads = np.zeros((N_CORES, n_blocks), dtype=np.int64)
    for c in range(N_CORES):
        s0 = c * segs_per_core
        s1 = min(s0 + segs_per_core, n_pdg)
        blocks, loads = _binpack_core(counts[s0:s1], n_blocks)
        core_blocks.append(blocks)
        core_loads[c] = loads
    tiles_per_block = np.maximum(
        1, (core_loads.max(axis=0) + P - 1) // P
    ).astype(int)
    tile_off = np.concatenate([[0], np.cumsum(tiles_per_block)]).astype(int)
    n_cols = int(tile_off[-1])
    seg_slots = n_blocks * P
    e_slots = n_cols * P

    in_maps = []
    out_seg = []  # per core: global seg id per out row (-1 = pad)
    for c in range(N_CORES):
        s0 = c * segs_per_core
        gidx_core = np.zeros(e_slots, dtype=np.int32)
        slid_core = np.full(e_slots, -1.0, dtype=x_np_dt)
        root_core = np.zeros(seg_slots, dtype=np.int32)
        oseg = np.full(seg_slots, -1, dtype=np.int64)
        for b, segs in enumerate(core_blocks[c]):
            o0 = tile_off[b] * P
            cap = tiles_per_block[b] * P
            pos = 0
            for j, sl in enumerate(segs):
                g = s0 + sl  # global segment id
                root_core[b * P + j] = root_full[g]
                oseg[b * P + j] = g
                e0, e1 = cum[g], cum[g + 1]
                n_e = e1 - e0
                if pos + n_e > cap:
                    raise OverflowError((c, b, pos, n_e, cap))
                gidx_core[o0 + pos : o0 + pos + n_e] = gid_sorted[e0:e1]
                slid_core[o0 + pos : o0 + pos + n_e] = float(j)
                pos += n_e
        # slot (col, p) at linear col*P+p -> DRAM layout [p, col] rows.
        gidx_core = np.ascontiguousarray(
            gidx_core.reshape(n_cols, P).T).ravel()
        slid_core = np.ascontiguousarray(
            slid_core.reshape(n_cols, P).T).ravel()
        root_core = np.ascontiguousarray(
            root_core.reshape(n_blocks, P).T).ravel()
        in_maps.append({
            "ast": ast_np,
            "wkt": wkt_s,
            "wv": wv_np,
            "gidx": gidx_core,
            "slid": slid_core,
            "root": root_core,
        })
        out_seg.append(oseg)

    meta = {
        "x_np_dt": x_np_dt,
        "n_ast": n_ast,
        "n_blocks": n_blocks,
        "tiles_per_block": tiles_per_block,
        "segs_per_core": segs_per_core,
        "n_pdg": n_pdg,
        "out_seg": out_seg,
        "n_tiles_total": n_cols,
    }
    return in_maps, meta


def _run(
    ast_np, wkt_s, wv_np, ast_to_pdg_key, ast_to_pdg_value,
    pdg_to_root_key, pdg_to_root_value, n_pdg,
    segs_per_core=None, n_blocks=None, trace=False,
):
    in_maps, meta = prepare_in_maps(
        ast_np, wkt_s, wv_np, ast_to_pdg_key, ast_to_pdg_value,
        pdg_to_root_key, pdg_to_root_value, n_pdg,
        segs_per_core=segs_per_core, n_blocks=n_blocks,
    )
    nc = _get_nc(meta["n_ast"], meta["tiles_per_block"], xdt=X_DT)
    res = run_bass_kernel_spmd(nc, in_maps, list(range(N_CORES)), trace=trace)

    full = np.zeros((n_pdg, D), dtype=np.float32)
    for c in range(N_CORES):
        oseg = meta["out_seg"][c]
        valid = oseg >= 0
        full[oseg[valid]] = res.results[c]["out"][valid]
    return full, res


def kernel(
    ast_nodes_encodings, Wk, Wv, ast_to_pdg_key, ast_to_pdg_value,
    pdg_to_root_key, pdg_to_root_value, nr_cfg_nodes,
):
    ast_np = np.ascontiguousarray(np.asarray(ast_nodes_encodings, dtype=np.float32))
    wk_np = np.asarray(Wk, dtype=np.float32)
    wv_np = np.ascontiguousarray(np.asarray(Wv, dtype=np.float32))
    scale = np.float32(1.0 / np.sqrt(ast_np.shape[1]))
    wkt_s = np.ascontiguousarray(wk_np.T * scale)

    n_pdg = int(nr_cfg_nodes)
    assert ast_np.shape == (N_AST_FULL, D) and n_pdg == N_PDG_FULL

    full, _ = _run(
        ast_np, wkt_s, wv_np,
        np.asarray(ast_to_pdg_key), np.asarray(ast_to_pdg_value),
        np.asarray(pdg_to_root_key), np.asarray(pdg_to_root_value),
        n_pdg,
    )
    return full


# revision 13
# speedup vs baseline: 1.8815x; 1.8815x over previous
"""Trainium2 Bass kernel for CFGSubASTExpressionCombiner (segment-softmax
attention over sub-ASTs grouped by PDG node).

Contract: kernel(**inputs) takes FULL unsharded numpy inputs, returns the
FULL [N_PDG, D] output. Internally shards PDG segments across 8 NeuronCores.
Within a core, segments are bin-packed (LPT) into 49 blocks of <=128
segments each so per-block element counts are flat; per-block element-tile
counts are baked into the program (variable, data-dependent).

Math (per segment s with element set E_s, all on device):
    q_s   = ast[root(s)]
    qk_s  = q_s @ (Wk.T * scale)        (scale folded into host-passed WkT)
    S[e,s]= x_e . qk_s                  (dense per 128-seg block, via PE)
    P     = exp(S) * [seg(e)==s]        (fused is_equal*exp on DVE)
    [U|Z] = P.T @ [X|1]                 (segment sums via PE, PSUM accum)
    out_s = (U_s / max(Z_s,eps)) @ Wv
No max-subtraction: scores are ~N(0,1) (|s|<~6), exp is safe in fp32, and
softmax is shift-invariant so results match the reference to fp32 rounding.

Engine budget per element tile (128 elements):
    Pool : 1 indirect gather (~1us SWDGE, the bottleneck)
    PE   : 2 transposes + 2 S-chunks + 1 U-accum  (~770 col-cycles)
    DVE  : xT copy (256) + fused mask*exp (128)
    Act  : exp (128) - Act engine does ONLY Exp (no act-table thrash)
"""

import math

import numpy as np

import concourse.bass as bass
import concourse.bacc as bacc
import concourse.mybir as mybir
import concourse.tile as tile
from concourse.bass_utils import run_bass_kernel_spmd
from concourse.masks import make_identity

P = 128
D = 256
N_CORES = 8

# Full-problem constants (hardcoded per contract).
N_AST_FULL = 500000
N_PDG_FULL = 50000
SEGS_PER_CORE_FULL = N_PDG_FULL // N_CORES          # 6250
N_BLOCKS_FULL = math.ceil(SEGS_PER_CORE_FULL / P)   # 49

f32 = mybir.dt.float32
i32 = mybir.dt.int32
bf16 = mybir.dt.bfloat16

# x-path dtype: flip both to bf16 to halve gather bytes (validated vs
# the 2e-2 rel-err gate before enabling).
try:
    import ml_dtypes
    _NP_BF16 = ml_dtypes.bfloat16
except ImportError:
    _NP_BF16 = None
X_DT = bf16
X_NP_DT = _NP_BF16 if _NP_BF16 is not None else np.float32
if _NP_BF16 is None:
    X_DT = f32

EXP = mybir.ActivationFunctionType.Exp


def _build_nc(n_ast, tiles_per_block, mode="full", reps=1,
              xp_bufs=10, blk_bufs=3, xdt=f32, windows=None):
    """One SPMD NeuronCore program. Sizes fixed at build time.

    tiles_per_block[b] = number of 128-element tiles in segment-block b.
    mode: "full" = real kernel; "gather" = gathers + tiny reduces only.
    reps: repeat the whole block loop (differential timing only).
    """
    n_blocks = len(tiles_per_block)
    tile_off = np.concatenate([[0], np.cumsum(tiles_per_block)]).astype(int)
    n_cols = int(tile_off[-1])
    seg_slots = n_blocks * P
    e_slots = n_cols * P

    nc = bacc.Bacc()
    ast = nc.declare_dram_parameter("ast", [n_ast, D], xdt, isOutput=False)
    wkt = nc.declare_dram_parameter("wkt", [D, D], xdt, isOutput=False)
    wv = nc.declare_dram_parameter("wv", [D, D], xdt, isOutput=False)
    gidx = nc.declare_dram_parameter("gidx", [e_slots], i32, isOutput=False)
    slid = nc.declare_dram_parameter("slid", [e_slots], xdt, isOutput=False)
    root = nc.declare_dram_parameter("root", [seg_slots], i32, isOutput=False)
    out = nc.declare_dram_parameter("out", [seg_slots, D], f32, isOutput=True)

    with tile.TileContext(nc) as tc:
        with (
            tc.tile_pool(name="const", bufs=1) as cpool,
            tc.tile_pool(name="blk", bufs=blk_bufs) as bpool,
            tc.tile_pool(name="xp", bufs=xp_bufs) as xpool,
            tc.tile_pool(name="pt", bufs=2, space="PSUM") as pt,
            tc.tile_pool(name="pu", bufs=2, space="PSUM") as pu,
        ):
            # Resident constants: Wk.T (pre-scaled) and Wv as two 128-row
            # K-chunks side by side; identity for PE transpose; iota row.
            wk2 = cpool.tile([P, 2 * D], xdt)
            nc.sync.dma_start(out=wk2[:, 0:D], in_=wkt[0:P, :])
            nc.sync.dma_start(out=wk2[:, D : 2 * D], in_=wkt[P : 2 * P, :])
            wv2 = cpool.tile([P, 2 * D], xdt)
            nc.sync.dma_start(out=wv2[:, 0:D], in_=wv[0:P, :])
            nc.sync.dma_start(out=wv2[:, D : 2 * D], in_=wv[P : 2 * P, :])
            ident = cpool.tile([P, P], xdt)
            make_identity(nc, ident[:])
            iota_i = cpool.tile([P, P], i32)
            nc.gpsimd.iota(iota_i[:], pattern=[[1, P]], base=0, channel_multiplier=0)
            iota_f = cpool.tile([P, P], xdt)
            nc.vector.tensor_copy(iota_f[:], iota_i[:])
            zmat = None
            if windows is not None:
                zmat = cpool.tile([P, P], xdt)
                nc.vector.memset(zmat[:], 0.0)

            # All index arrays resident in SBUF, one DMA each: column c of
            # gx_all/sl_all is element-tile c (tile t of block b at
            # c = tile_off[b] + t); column b of root_all is segment block b.
            gx_all = cpool.tile([P, n_cols], i32)
            nc.sync.dma_start(
                out=gx_all[:], in_=gidx[:].rearrange("(p c) -> p c", c=n_cols)
            )
            sl_all = cpool.tile([P, n_cols], xdt)
            nc.sync.dma_start(
                out=sl_all[:], in_=slid[:].rearrange("(p c) -> p c", c=n_cols)
            )
            root_all = cpool.tile([P, n_blocks], i32)
            nc.sync.dma_start(
                out=root_all[:], in_=root[:].rearrange("(p b) -> p b", b=n_blocks)
            )

            for _rep in range(reps):
              for b in range(n_blocks):
                t_b = int(tiles_per_block[b])
                # ---- segment side: q rows -> qk^T (d on partitions) ----
                q = bpool.tile([P, D], xdt)
                nc.gpsimd.indirect_dma_start(
                    out=q[:],
                    out_offset=None,
                    in_=ast[:],
                    in_offset=bass.IndirectOffsetOnAxis(ap=root_all[:, b : b + 1], axis=0),
                )
                if mode == "gather":
                    acc = xpool.tile([P, 16], f32, tag="acc")
                    nc.vector.tensor_reduce(
                        acc[:, 15:16], q[:],
                        axis=mybir.AxisListType.X, op=mybir.AluOpType.max,
                    )
                    for t in range(t_b):
                        c = tile_off[b] + t
                        x = xpool.tile([P, D], xdt, tag="xg")
                        nc.gpsimd.indirect_dma_start(
                            out=x[:],
                            out_offset=None,
                            in_=ast[:],
                            in_offset=bass.IndirectOffsetOnAxis(
                                ap=gx_all[:, c : c + 1], axis=0
                            ),
                        )
                        nc.vector.tensor_reduce(
                            acc[:, t % 15 : t % 15 + 1], x[:],
                            axis=mybir.AxisListType.X, op=mybir.AluOpType.max,
                        )
                    nc.sync.dma_start(
                        out=out[b * P : (b + 1) * P, 0:16], in_=acc[:]
                    )
                    continue
                qT_ps = pt.tile([P, D], xdt, tag="tr")
                nc.tensor.transpose(qT_ps[:, 0:P], q[:, 0:P], ident[:])
                nc.tensor.transpose(qT_ps[:, P:D], q[:, P:D], ident[:])
                qT = bpool.tile([P, D], xdt)
                nc.vector.tensor_copy(qT[:], qT_ps[:])

                qkT_ps = pt.tile([P, D], f32, tag="mm")
                for m in range(2):
                    for k in range(2):
                        nc.tensor.matmul(
                            qkT_ps[:, m * P : (m + 1) * P],
                            lhsT=wk2[:, k * D + m * P : k * D + (m + 1) * P],
                            rhs=qT[:, k * P : (k + 1) * P],
                            start=(k == 0),
                            stop=(k == 1),
                        )
                qkT = bpool.tile([P, D], xdt)
                nc.vector.tensor_copy(qkT[:], qkT_ps[:])

                # ---- element side: accumulate [U | Z] over t_b tiles ----
                u_ps = pu.tile([P, D + 1], f32, tag="u")
                if windows is not None:
                    # zero-init the full U region (windowed tiles only touch
                    # their own partition ranges)
                    nc.tensor.matmul(
                        u_ps[:],
                        lhsT=zmat[:],
                        rhs=wk2[:, 0 : D + 1],
                        start=True,
                        stop=False,
                        skip_group_check=True,
                    )
                for t in range(t_b):
                    c = tile_off[b] + t
                    lo, w = windows[c] if windows is not None else (0, P)
                    xt_ = xpool.tile([P, D + 1], xdt, tag="xg")
                    x = xt_[:]
                    nc.vector.memset(x[:, D : D + 1], 1.0)
                    nc.gpsimd.indirect_dma_start(
                        out=x[:, 0:D],
                        out_offset=None,
                        in_=ast[:],
                        in_offset=bass.IndirectOffsetOnAxis(
                            ap=gx_all[:, c : c + 1], axis=0
                        ),
                    )
                    xT_ps = pt.tile([P, D], xdt, tag="tr")
                    nc.tensor.transpose(xT_ps[:, 0:P], x[:, 0:P], ident[:])
                    nc.tensor.transpose(xT_ps[:, P:D], x[:, P:D], ident[:])
                    xT = xpool.tile([P, D], xdt)
                    nc.vector.tensor_copy(xT[:], xT_ps[:])

                    s_ps = pt.tile([P, P], f32, tag="s")
                    for k in range(2):
                        nc.tensor.matmul(
                            s_ps[:, 0:w],
                            lhsT=xT[:, k * P : (k + 1) * P],
                            rhs=qkT[:, k * P + lo : k * P + lo + w],
                            start=(k == 0),
                            stop=(k == 1),
                        )
                    ptil = xpool.tile([P, P], xdt)
                    nc.scalar.activation(ptil[:, 0:w], s_ps[:, 0:w], EXP)
                    pmat = xpool.tile([P, P], xdt)
                    nc.vector.scalar_tensor_tensor(
                        out=pmat[:, 0:w],
                        in0=iota_f[:, 0:w],
                        scalar=sl_all[:, c : c + 1],
                        in1=ptil[:, 0:w],
                        op0=mybir.AluOpType.is_equal,
                        op1=mybir.AluOpType.mult,
                    )
                    if windows is not None:
                        nc.tensor.matmul(
                            u_ps[lo : lo + w, :],
                            lhsT=pmat[:, 0:w],
                            rhs=x[:],
                            start=False,
                            stop=(t == t_b - 1),
                            skip_group_check=True,
                        )
                    else:
                        nc.tensor.matmul(
                            u_ps[:],
                            lhsT=pmat[:],
                            rhs=x[:],
                            start=(t == 0),
                            stop=(t == t_b - 1),
                        )

                # ---- finalize block: (U/Z) @ Wv  (DVE only; Act stays Exp) --
                z = bpool.tile([P, 1], f32)
                nc.vector.tensor_scalar_max(z[:], u_ps[:, D : D + 1], 1e-30)
                rz = bpool.tile([P, 1], f32)
                nc.vector.reciprocal(rz[:], z[:])
                up = bpool.tile([P, D], xdt)
                nc.vector.tensor_scalar_mul(up[:], u_ps[:, 0:D], rz[:, 0:1])
                upT_ps = pt.tile([P, D], xdt, tag="tr")
                nc.tensor.transpose(upT_ps[:, 0:P], up[:, 0:P], ident[:])
                nc.tensor.transpose(upT_ps[:, P:D], up[:, P:D], ident[:])
                upT = bpool.tile([P, D], xdt)
                nc.vector.tensor_copy(upT[:], upT_ps[:])
                f_ps = pt.tile([P, D], f32, tag="mm")
                for k in range(2):
                    nc.tensor.matmul(
                        f_ps[:],
                        lhsT=upT[:, k * P : (k + 1) * P],
                        rhs=wv2[:, k * D : (k + 1) * D],
                        start=(k == 0),
                        stop=(k == 1),
                    )
                o = bpool.tile([P, D], f32)
                nc.vector.tensor_copy(o[:], f_ps[:])
                nc.sync.dma_start(out=out[b * P : (b + 1) * P, :], in_=o[:])
    nc.finalize()
    return nc


_NC_CACHE = {}


def _get_nc(n_ast, tiles_per_block, mode="full", reps=1, xp_bufs=10,
            blk_bufs=3, xdt=f32, windows=None):
    key = (n_ast, tuple(tiles_per_block), mode, reps, xp_bufs, blk_bufs,
           str(xdt), tuple(windows) if windows is not None else None)
    if key not in _NC_CACHE:
        _NC_CACHE[key] = _build_nc(
            n_ast, list(tiles_per_block), mode=mode, reps=reps,
            xp_bufs=xp_bufs, blk_bufs=blk_bufs, xdt=xdt, windows=windows,
        )
    return _NC_CACHE[key]


def _binpack_core(counts_core, n_blocks, caps=None):
    """Bin-pack segments into n_blocks blocks of <=128 segs each.

    caps: optional per-block element capacities (desc order). Segments go
    to the fullest block that still fits (best-fit decreasing), which packs
    tight blocks first so the shared cross-core tiles_per_block profile is
    achievable by every core. Falls back to LPT balance when caps is None.

    Returns (blocks, loads): blocks = list of lists of local segment ids,
    loads = element count per block. Blocks sorted by load descending so
    block index -> load rank is aligned across cores.
    """
    import heapq

    n_seg = len(counts_core)
    order = np.argsort(-counts_core, kind="stable")
    blocks = [[] for _ in range(n_blocks)]
    loads = np.zeros(n_blocks, dtype=np.int64)
    if caps is not None:
        caps = np.asarray(caps)
        # worst-fit by remaining capacity ratio keeps all blocks feasible;
        # max-heap on remaining capacity
        heap = [(-caps[b], b) for b in range(n_blocks)]
        heapq.heapify(heap)
        for s in order:
            load, b = heapq.heappop(heap)
            rem = -load
            n_e = counts_core[s]
            if n_e > rem:
                raise OverflowError((b, n_e, rem))
            blocks[b].append(s)
            loads[b] += n_e
            if len(blocks[b]) < P:
                heapq.heappush(heap, (-(rem - n_e), b))
        border = np.argsort(-loads, kind="stable")
        blocks = [blocks[i] for i in border]
        loads = loads[border]
        inter = []
        for segs in blocks:
            segs = sorted(segs, key=lambda s: -counts_core[s])
            out, i, j = [], 0, len(segs) - 1
            while i <= j:
                out.append(segs[i])
                if i < j:
                    out.append(segs[j])
                i += 1
                j -= 1
            inter.append(out)
        return inter, loads
    heap = [(0, b) for b in range(n_blocks)]
    heapq.heapify(heap)
    deferred = []
    for s in order:
        load, b = heapq.heappop(heap)
        blocks[b].append(s)
        loads[b] = load + counts_core[s]
        if len(blocks[b]) < P:
            heapq.heappush(heap, (loads[b], b))
        else:
            deferred.append(b)
    # sort blocks by load desc for cross-core alignment
    border = np.argsort(-loads, kind="stable")
    blocks = [blocks[i] for i in border]
    loads = loads[border]
    # big/small interleave within each block: keeps the number of
    # segments in any 128-element run near the block average, which
    # tightens the per-tile segment windows
    inter = []
    for segs in blocks:
        segs = sorted(segs, key=lambda s: -counts_core[s])
        out, i, j = [], 0, len(segs) - 1
        while i <= j:
            out.append(segs[i])
            if i < j:
                out.append(segs[j])
            i += 1
            j -= 1
        inter.append(out)
    return inter, loads


def prepare_in_maps(
    ast_np, wkt_s, wv_np, ast_to_pdg_key, ast_to_pdg_value,
    pdg_to_root_key, pdg_to_root_value, n_pdg,
    segs_per_core=None, n_blocks=None, x_np_dt=None,
):
    """Host-side prep: sort elements by segment, bin-pack segments into
    blocks (per core), pad, build per-core in_maps.

    Returns (in_maps, meta) where meta["tiles_per_block"] parameterizes the
    program and meta["out_seg"] maps per-core out rows -> global segment ids.
    """
    n_ast = ast_np.shape[0]
    if x_np_dt is None:
        x_np_dt = X_NP_DT
    if ast_np.dtype != x_np_dt:
        ast_np = np.ascontiguousarray(ast_np.astype(x_np_dt))
        wkt_s = np.ascontiguousarray(wkt_s.astype(x_np_dt))
        wv_np = np.ascontiguousarray(wv_np.astype(x_np_dt))
    if segs_per_core is None:
        segs_per_core = (n_pdg + N_CORES - 1) // N_CORES
    if n_blocks is None:
        n_blocks = math.ceil(segs_per_core / P)

    order = np.argsort(ast_to_pdg_value, kind="stable")
    seg_sorted = np.asarray(ast_to_pdg_value)[order]
    gid_sorted = np.asarray(ast_to_pdg_key)[order].astype(np.int32)
    counts = np.bincount(seg_sorted, minlength=n_pdg).astype(np.int64)
    cum = np.concatenate([[0], np.cumsum(counts)]).astype(np.int64)

    root_full = np.zeros(n_pdg, dtype=np.int32)
    root_full[np.asarray(pdg_to_root_key)] = np.asarray(pdg_to_root_value)

    # Per-core bin-packing; then per-block-index max tiles across cores
    # (the SPMD program is shared by all cores).
    # Shared tiles-per-block profile: smallest tile budget with ~5%
    # slack over the biggest core, split into ceil/floor capacity classes.
    core_E = np.array([
        int(counts[c * segs_per_core : min((c + 1) * segs_per_core, n_pdg)]
            .sum()) for c in range(N_CORES)
    ])
    need = int(core_E.max() * 1.02) + 2 * P
    base = need // n_blocks // P
    n_hi = min(n_blocks,
               math.ceil((need - n_blocks * base * P) / P))
    tiles_per_block = np.array(
        [base + 1] * n_hi + [base] * (n_blocks - n_hi), dtype=int)
    caps = tiles_per_block * P
    core_blocks = []
    for c in range(N_CORES):
        s0 = c * segs_per_core
        s1 = min(s0 + segs_per_core, n_pdg)
        try:
            blocks, _ = _binpack_core(counts[s0:s1], n_blocks, caps=caps)
        except OverflowError:
            # tight profile infeasible for this core: bump every block
            tiles_per_block = tiles_per_block + 1
            caps = tiles_per_block * P
            blocks, _ = _binpack_core(counts[s0:s1], n_blocks, caps=caps)
        core_blocks.append(blocks)
    tile_off = np.concatenate([[0], np.cumsum(tiles_per_block)]).astype(int)
    n_cols = int(tile_off[-1])
    seg_slots = n_blocks * P
    e_slots = n_cols * P

    in_maps = []
    out_seg = []  # per core: global seg id per out row (-1 = pad)
    core_arrays = []
    win_lo = np.full(n_cols, P, dtype=np.int64)   # cross-core min
    win_hi = np.zeros(n_cols, dtype=np.int64)     # cross-core max
    for c in range(N_CORES):
        s0 = c * segs_per_core
        gidx_core = np.zeros(e_slots, dtype=np.int32)
        slid_core = np.full(e_slots, -1.0, dtype=np.float64)
        root_core = np.zeros(seg_slots, dtype=np.int32)
        oseg = np.full(seg_slots, -1, dtype=np.int64)
        for b, segs in enumerate(core_blocks[c]):
            o0 = tile_off[b] * P
            cap = tiles_per_block[b] * P
            pos = 0
            for j, sl in enumerate(segs):
                g = s0 + sl  # global segment id
                root_core[b * P + j] = root_full[g]
                oseg[b * P + j] = g
                e0, e1 = cum[g], cum[g + 1]
                n_e = e1 - e0
                if pos + n_e > cap:
                    raise OverflowError((c, b, pos, n_e, cap))
                gidx_core[o0 + pos : o0 + pos + n_e] = gid_sorted[e0:e1]
                slid_core[o0 + pos : o0 + pos + n_e] = float(j)
                pos += n_e
        # per-tile local-segment windows (for the windowed program)
        sl2 = slid_core.reshape(n_cols, P)
        for t in range(n_cols):
            v = sl2[t][sl2[t] >= 0]
            if len(v):
                win_lo[t] = min(win_lo[t], int(v.min()))
                win_hi[t] = max(win_hi[t], int(v.max()) + 1)
        core_arrays.append((gidx_core, slid_core, root_core))
        out_seg.append(oseg)

    # PE PSUM writes at non-zero base partitions are limited to 32
    # partitions; keep windows base-0 (narrowing still trims the
    # S-matmul stream, exp, and mask to [0, hi))
    win_lo = np.zeros_like(win_lo)
    win_w = np.maximum(win_hi, 1)
    windows = tuple((int(win_lo[t]), int(win_w[t])) for t in range(n_cols))

    for c in range(N_CORES):
        gidx_core, slid_core, root_core = core_arrays[c]
        # make slid relative to the tile's window start
        sl2 = slid_core.reshape(n_cols, P)
        for t in range(n_cols):
            m = sl2[t] >= 0
            sl2[t][m] -= win_lo[t]
        slid_core = sl2.ravel().astype(x_np_dt)
        # slot (col, p) at linear col*P+p -> DRAM layout [p, col] rows.
        gidx_core = np.ascontiguousarray(
            gidx_core.reshape(n_cols, P).T).ravel()
        slid_core = np.ascontiguousarray(
            slid_core.reshape(n_cols, P).T).ravel()
        root_core = np.ascontiguousarray(
            root_core.reshape(n_blocks, P).T).ravel()
        in_maps.append({
            "ast": ast_np,
            "wkt": wkt_s,
            "wv": wv_np,
            "gidx": gidx_core,
            "slid": slid_core,
            "root": root_core,
        })

    meta = {
        "windows": windows,
        "x_np_dt": x_np_dt,
        "n_ast": n_ast,
        "n_blocks": n_blocks,
        "tiles_per_block": tiles_per_block,
        "segs_per_core": segs_per_core,
        "n_pdg": n_pdg,
        "out_seg": out_seg,
        "n_tiles_total": n_cols,
    }
    return in_maps, meta


def _run(
    ast_np, wkt_s, wv_np, ast_to_pdg_key, ast_to_pdg_value,
    pdg_to_root_key, pdg_to_root_value, n_pdg,
    segs_per_core=None, n_blocks=None, trace=False,
):
    in_maps, meta = prepare_in_maps(
        ast_np, wkt_s, wv_np, ast_to_pdg_key, ast_to_pdg_value,
        pdg_to_root_key, pdg_to_root_value, n_pdg,
        segs_per_core=segs_per_core, n_blocks=n_blocks,
    )
    nc = _get_nc(meta["n_ast"], meta["tiles_per_block"], xdt=X_DT,
                 windows=meta["windows"])
    res = run_bass_kernel_spmd(nc, in_maps, list(range(N_CORES)), trace=trace)

    full = np.zeros((n_pdg, D), dtype=np.float32)
    for c in range(N_CORES):
        oseg = meta["out_seg"][c]
        valid = oseg >= 0
        full[oseg[valid]] = res.results[c]["out"][valid]
    return full, res


def kernel(
    ast_nodes_encodings, Wk, Wv, ast_to_pdg_key, ast_to_pdg_value,
    pdg_to_root_key, pdg_to_root_value, nr_cfg_nodes,
):
    ast_np = np.ascontiguousarray(np.asarray(ast_nodes_encodings, dtype=np.float32))
    wk_np = np.asarray(Wk, dtype=np.float32)
    wv_np = np.ascontiguousarray(np.asarray(Wv, dtype=np.float32))
    scale = np.float32(1.0 / np.sqrt(ast_np.shape[1]))
    wkt_s = np.ascontiguousarray(wk_np.T * scale)

    n_pdg = int(nr_cfg_nodes)
    assert ast_np.shape == (N_AST_FULL, D) and n_pdg == N_PDG_FULL

    full, _ = _run(
        ast_np, wkt_s, wv_np,
        np.asarray(ast_to_pdg_key), np.asarray(ast_to_pdg_value),
        np.asarray(pdg_to_root_key), np.asarray(pdg_to_root_value),
        n_pdg,
    )
    return full
